# revision 88
# baseline (speedup 1.0000x reference)
"""Trainium2 Bass kernel for the phase-type log-prior problem.

reference(w, S, alpha) = sum_m log( alpha^T expm(w_m * S) s ),  s = -S @ 1

Since S is a fixed matrix and w_m are scalars, expm(w_m S) = V diag(exp(w_m d)) V^-1
with (d, V) the eigendecomposition of S (real eigenvalues for the lower-triangular
phase-type sub-generator this problem uses).  Hence

    density[m] = sum_i c_i * exp(d_i * w_m),   c = (alpha^T V) * (V^-1 s)

Two accuracy-preserving reductions make the device kernel tiny:
1. Exponential sums are extremely compressible: a least-squares refit on
   the actual w-range represents the 8-term density with 2 terms at ~1e-5
   pointwise relative error whose mean residual is ~1e-12 (so the summed
   result is exact to ~1e-8).  Validated against tolerance at runtime,
   with automatic fallback to more terms / the exact decomposition.
2. Factoring out the dominant term c_k e^{d_k w} makes the log-density
       ln c_k + d_k w + ln(1 + sum_ratios),
   where the affine part is an exact fp64 host reduction over the fp32 w
   (M ln c_k + d_k * sum(w)) and the +1 rides the Ln activation's free
   per-partition bias.  The device computes only the tiny residual, so
   quantizing the on-device w to bf16 (halving the input DMA) costs ~1e-11.

3. The factored residual ln(1 + r e^{dw}) is itself nearly exponential, so
   the preferred fit is the fully-fused form
       log dens(w) ~ a + b*w - e^{p + q*w}
   (runtime-validated to ~1e-5 pointwise / ~1e-7 mean; graceful fallback
   to the n-term exp+Ln path, then to the exact decomposition, then to a
   host Pade expm).  The device then computes ONE Exp activation per
   element.

Per core (raw Bass, direct emission, no Tile): bf16 w shard [128, J] split
into two DMAs across the SP/Activation HWDGE engines; bias constants as
GPSIMD memsets (compile-time immediates); one Exp activation with
per-partition accum_out; a PE ones-vector matmul reduces the 128 partials
to one scalar; a single 4-byte DMA returns it.  A dummy activation at
program start hoists the ~1.5us activation-table load over the input DMA;
the Bass init barrier / const pool is stripped, and walrus'
1-wait-per-instruction ISA limit is respected via standalone NoOp waits.
Data-parallel over 8 cores; the host combines the 8 scalars with the
exact affine part in fp64 and removes the known padding contribution.
"""

import os
import sys

import ml_dtypes
import numpy as np

sys.path.insert(0, "/opt/trn_rl_repo")

import concourse.bass as bass  # noqa: E402
import concourse.mybir as mybir  # noqa: E402
from concourse.bass_utils import run_bass_kernel_spmd  # noqa: E402

N_CORES = 8
F32 = mybir.dt.float32
AF = mybir.ActivationFunctionType

_program_cache: dict = {}
_last_results = None


def _build_program(d: np.ndarray, logc: np.ndarray, n_neg: int, P: int, J: int,
                   ln_bias_one: bool = False, ln_scale: float = 1.0):
    """Raw-Bass SPMD program, direct emission (no Block, no tail barrier).

    d, logc: per-term exp scale / bias ln|c_i|, NEGATIVE c terms first (so the
    negative add-chain finishes early and only the positive chain's last add
    plus the subtract trail the final exp).
    P, J: per-core tile layout [P partitions, J free]; shard size = P*J.
    Padding is handled on the host (pad value 1.0; its known log-density is
    subtracted from the total), so the device treats every element as real.
    The packed input row layout is: J w-values, n_terms biases, one 0.0
    (used as the Ln bias so no engine depends on the init-time const pool,
    which we strip below along with the init barrier).
    """
    n_terms = len(d)
    n_pos = n_terms - n_neg
    assert n_pos >= 1

    nc = bass.Bass()
    BF16 = mybir.dt.bfloat16
    wb_in = nc.declare_dram_parameter("wbf", [P, J], BF16, isOutput=False)
    out = nc.declare_dram_parameter("partials", [1, 1], F32, isOutput=True)

    with (
        nc.sbuf_tensor([P, J], BF16) as WBt,
        nc.sbuf_tensor([P, n_terms + 2], F32) as Bt,
        nc.sbuf_tensor([P, n_terms * J], F32) as Ft,
        nc.sbuf_tensor([P, J], F32) as accp_t,
        nc.sbuf_tensor([P, J], F32) as accn_t,
        nc.sbuf_tensor([P, J], F32) as logd_t,
        nc.sbuf_tensor([P, 2], F32) as scratch_t,
        nc.psum_tensor([1, 1], F32) as psum_t,
        nc.semaphore("s_in") as s_in,
        nc.semaphore("s_act") as s_act,
        nc.semaphore("s_dve") as s_dve,
        nc.semaphore("s_pe") as s_pe,
        nc.semaphore("s_out") as s_out,
        nc.semaphore("s_pool") as s_pool,
    ):
        WB = WBt[:]
        BIAS = Bt[:]
        F = Ft[:]
        accp = accp_t[:]
        accn = accn_t[:]
        logd = logd_t[:]
        scratch = scratch_t[:]
        W = WB[:, 0:J]
        B = BIAS[:, 0:n_terms]
        zbias = BIAS[:, n_terms:n_terms + 1]
        ones = BIAS[:, n_terms + 1:n_terms + 2]
        Fi = [F[:, i * J:(i + 1) * J] for i in range(n_terms)]
        part = scratch[:, 0:1]
        res_sb = scratch[0:1, 1:2]
        psum = psum_t[:]

        # --- Input DMA (bf16 w, half the bytes) split in halves across both
        # HWDGE engines (SP + Activation) so the pieces stream in parallel.
        H = P // 2
        nc.sync.dma_start(WB[0:H, :], wb_in[0:H, :]).then_inc(s_in, 16)
        nc.scalar.dma_start(WB[H:P, :], wb_in[H:P, :]).then_inc(s_in, 16)

        # --- Pool: bias columns ln|c_i|, 0.0 (Ln bias), 1.0 (sum weights).
        # Compile-time immediates written by the otherwise-idle GPSIMD; a
        # [128,small] fp32 DMA would cost 128 serial descriptors instead. ---
        for i in range(n_terms):
            nc.gpsimd.memset(BIAS[:, i:i + 1], float(logc[i]))
        nc.gpsimd.memset(zbias, 0.0)
        nc.gpsimd.memset(ones, 1.0).then_inc(s_pool, 1)

        # --- Scalar: dummy exp hoists the act-table load over the DMA ---
        nc.scalar.activation(scratch[0:1, 1:2], scratch[0:1, 0:1], AF.Exp,
                             bias=scratch[0:1, 0:1], scale=1.0).then_inc(s_act, 1)
        nc.scalar.wait_ge(s_in, 32)
        nc.scalar.wait_ge(s_pool, 1)
        for i in range(n_terms):
            nc.scalar.activation(
                Fi[i], W, AF.Exp, bias=B[:, i:i + 1], scale=float(d[i]),
            ).then_inc(s_act, 1)

        # --- DVE: add/sub chain trailing the exps (F_i ready at
        # s_act >= i+2); the last chain op is the only DVE work after the
        # final exp. ---
        n_dve = 0

        def emit_chain(acc, base, count):
            nonlocal n_dve
            if count == 1:
                return Fi[base]
            nc.vector.wait_ge(s_act, base + 1 + 2)
            nc.vector.tensor_add(acc, Fi[base], Fi[base + 1]).then_inc(s_dve, 1)
            n_dve += 1
            for k in range(2, count):
                nc.vector.wait_ge(s_act, base + k + 2)
                nc.vector.tensor_add(acc, acc, Fi[base + k]).then_inc(s_dve, 1)
                n_dve += 1
            return acc

        # neg chain: accn = F0+..+F_{nn-1}; then ONE early type-switching
        # subtract accp = F_nn - accn; then pure ADDs accp += F_k.  The tail
        # after the final exp is a single ADD whose predecessor is also an
        # ADD (uop-table switches between TENSOR_TENSOR ALU ops cost a
        # ~0.5us pipe reconfig drain; keep them off the critical tail).
        dens = accp
        if n_terms == 1:
            dens = Fi[0]
        elif n_neg == 0:
            emit_chain(accp, 0, n_terms)
        else:
            neg_ap = Fi[0] if n_neg == 1 else emit_chain(accn, 0, n_neg)
            nc.vector.wait_ge(s_act, n_neg + 2)        # F_{nn} ready
            nc.vector.tensor_sub(accp, Fi[n_neg], neg_ap).then_inc(s_dve, 1)
            n_dve += 1
            for k in range(n_neg + 1, n_terms):
                nc.vector.wait_ge(s_act, k + 2)
                nc.vector.tensor_add(accp, accp, Fi[k]).then_inc(s_dve, 1)
                n_dve += 1
        n_dve_ops = n_dve

        # --- Scalar: Ln with per-partition accumulation.  In the factored
        # form (ln_bias_one) this computes ln(ln_scale*x + 1). ---
        if n_dve_ops:
            nc.scalar.wait_ge(s_dve, n_dve_ops)
        lb = ones if ln_bias_one else zbias
        nc.scalar.activation(logd, dens, AF.Ln, bias=lb, scale=float(ln_scale),
                             accum_out=part).then_inc(s_act, 1)

        # --- PE: reduce the 128 per-partition sums to one value.  This keeps
        # the output DMA a single 4-byte descriptor; a [128,1] DMA costs ~8us
        # in serial HWDGE descriptor processing. ---
        nc.tensor.wait_ge(s_act, n_terms + 2)
        nc.tensor.matmul(psum[0:1, 0:1], ones, part,
                         start=True, stop=True).then_inc(s_pe, 1)
        # PSUM -> SBUF copy and the 4-byte output DMA both on Scalar
        # (HWDGE-capable): program order replaces two cross-engine semaphore
        # hops, and Sync then reaches the exit barrier right after the input
        # DMAs, so the wave-1 barrier gate is Scalar's copy+DMA instead of
        # a longer Sync chain.  No completion wait on s_out: the walrus
        # postamble (sem-clear storm + exit barrier, ~6us) runs long past
        # the 4-byte transfer.
        nc.scalar.wait_ge(s_pe, 1)
        nc.scalar.copy(res_sb, psum[0:1, 0:1])
        nc.scalar.dma_start(out[:], res_sb).then_inc(s_out, 16)

    _strip_init_overhead(nc)
    _hoist_dma_before_regmoves(nc)
    _split_multiwait(nc)
    return nc


def _hoist_dma_before_regmoves(nc):
    """Move each HWDGE engine's leading input DMAs (and the wait-free dummy
    activation that triggers the act-table load) in front of that engine's
    register-init moves (R8..R13 constants, unused by either) so transfers
    and the table load start a few hundred ns earlier."""
    for fn in nc.m.functions:
        for blk in fn.blocks:
            insts = blk.instructions
            for eng in (mybir.EngineType.SP, mybir.EngineType.Activation):
                first_mov = None
                dmas = []
                for idx, inst in enumerate(insts):
                    if inst.engine != eng:
                        continue
                    if isinstance(inst, mybir.InstRegisterMove):
                        if first_mov is None:
                            first_mov = idx
                        continue
                    if isinstance(inst, mybir.InstDMACopy):
                        dmas.append(idx)
                        continue
                    if isinstance(inst, mybir.InstActivation):
                        si = getattr(inst, "sync_info", None)
                        if si is None or not si.on_wait:
                            dmas.append(idx)   # the wait-free dummy act
                            continue
                    break  # engine's leading region ends at any other inst
                if first_mov is None or not dmas:
                    continue
                dmas = [i for i in dmas if i > first_mov]
                for k, idx in enumerate(dmas):
                    inst = insts.pop(idx)
                    insts.insert(first_mov + k, inst)


def _build_program_softplus(p: float, q: float, P: int, J: int):
    """Minimal program for the exp-corrected affine form
        log dens(w) = a + b*w - e^{p + q*w}
    (a, b handled exactly on host; the device sum is subtracted there).
    Per element the device computes ONE Exp activation (bias p rides in the
    activation's per-partition bias AP, scale q as immediate) with
    per-partition accumulation, then the ones-matmul partition reduction
    and the 4-byte result DMA."""
    nc = bass.Bass()
    BF16 = mybir.dt.bfloat16
    wb_in = nc.declare_dram_parameter("wbf", [P, J], BF16, isOutput=False)
    out = nc.declare_dram_parameter("partials", [1, 1], F32, isOutput=True)

    with (
        nc.sbuf_tensor([P, J], BF16) as WBt,
        nc.sbuf_tensor([P, 2], F32) as Bt,
        nc.sbuf_tensor([P, J], F32) as logd_t,
        nc.sbuf_tensor([P, 2], F32) as scratch_t,
        nc.psum_tensor([1, 1], F32) as psum_t,
        nc.semaphore("s_in") as s_in,
        nc.semaphore("s_act") as s_act,
        nc.semaphore("s_pe") as s_pe,
        nc.semaphore("s_out") as s_out,
        nc.semaphore("s_pool") as s_pool,
    ):
        W = WBt[:]
        pbias = Bt[:, 0:1]
        ones = Bt[:, 1:2]
        logd = logd_t[:]
        scratch = scratch_t[:]
        part = scratch[:, 0:1]
        res_sb = scratch[0:1, 1:2]
        psum = psum_t[:]

        H = P // 2
        nc.sync.dma_start(W[0:H, :], wb_in[0:H, :]).then_inc(s_in, 16)
        nc.scalar.dma_start(W[H:P, :], wb_in[H:P, :]).then_inc(s_in, 16)

        nc.gpsimd.memset(pbias, float(p))
        nc.gpsimd.memset(ones, 1.0).then_inc(s_pool, 1)

        # Dummy activation hoists the exp table load over the input DMA.
        nc.scalar.activation(scratch[0:1, 1:2], scratch[0:1, 0:1], AF.Exp,
                             bias=scratch[0:1, 0:1], scale=1.0).then_inc(s_act, 1)
        nc.scalar.wait_ge(s_in, 32)
        nc.scalar.wait_ge(s_pool, 1)
        nc.scalar.activation(logd, W, AF.Exp, bias=pbias, scale=float(q),
                             accum_out=part).then_inc(s_act, 1)

        nc.tensor.wait_ge(s_act, 2)
        nc.tensor.matmul(psum[0:1, 0:1], ones, part,
                         start=True, stop=True).then_inc(s_pe, 1)
        nc.scalar.wait_ge(s_pe, 1)
        nc.scalar.copy(res_sb, psum[0:1, 0:1])
        nc.scalar.dma_start(out[:], res_sb).then_inc(s_out, 16)

    _strip_init_overhead(nc)
    _hoist_dma_before_regmoves(nc)
    _split_multiwait(nc)
    return nc


def _fit_softplus(c, d, wmin, wmax, tol=1e-5, mean_tol=1e-7):
    """Fit log(sum c_i e^{d_i w}) ~ a + b w - e^{p + q w} on the range.
    Returns (a, b, p, q) or None if the fit misses tolerance."""
    if wmax - wmin < 1e-9:
        return None
    try:
        from scipy.optimize import least_squares
    except ImportError:
        return None
    grid = np.linspace(wmin, wmax, 3001)
    dens = np.exp(np.outer(grid, d)) @ c
    if dens.min() <= 0 or not np.isfinite(dens).all():
        return None
    logdens = np.log(dens)
    k = int(np.argmax(c))
    if c[k] <= 0:
        return None
    # init from factoring the dominant term; residual magnitude seeds p
    resid0 = 1.0 - dens / (c[k] * np.exp(d[k] * grid))
    m0 = max(float(np.abs(resid0).mean()), 1e-12)
    x0 = np.array([np.log(c[k]), d[k], np.log(m0), 0.0])

    def err(x):
        return x[0] + x[1] * grid - np.exp(x[2] + x[3] * grid) - logdens

    try:
        r = least_squares(err, x0, method="lm", max_nfev=50000)
    except Exception:
        return None
    rr = err(r.x)
    a, b, p, q = (float(v) for v in r.x)
    args = (p + q * grid)
    if (np.abs(rr).max() <= tol and abs(rr.mean()) <= mean_tol
            and np.isfinite(r.x).all() and args.max() < 5 and args.min() > -60):
        return a, b, p, q
    return None


def _strip_init_overhead(nc):
    """Remove the Bass-init const-pool memsets and the init all-engine
    barrier.  Nothing in the program reads the const APs (the Ln bias comes
    from the packed input instead), so the barrier that orders them is dead
    weight (~1.5us of preamble)."""
    for fn in nc.m.functions:
        for blk in fn.blocks:
            kept = []
            for inst in blk.instructions:
                if isinstance(inst, mybir.InstMemset):
                    outs = inst.outs
                    name = ""
                    try:
                        name = outs[0].memorylocation.name
                    except Exception:
                        try:
                            name = outs[0].tensor.name
                        except Exception:
                            pass
                    if str(name).startswith("const-"):
                        continue
                if isinstance(inst, (mybir.InstDrain, mybir.InstEventSemaphore)):
                    si = getattr(inst, "sync_info", None)
                    refs = []
                    if si is not None:
                        refs = [w.ant_name for w in si.on_wait] + \
                               [u.ant_name for u in si.on_update]
                    if refs and all(str(r).startswith("barrier_") for r in refs):
                        continue
                kept.append(inst)
            blk.instructions[:] = kept


def _split_multiwait(nc, limit: int = 1):
    """walrus rejects instructions whose embedded sync-wait list exceeds the
    engine ISA struct's slots (1 for Activation, ~3 for Drain).  Hoist excess
    waits into standalone NoOps on the same engine just before the
    instruction."""
    k = 0
    for fn in nc.m.functions:
        for blk in fn.blocks:
            new = []
            for inst in blk.instructions:
                si = getattr(inst, "sync_info", None)
                if si is not None and si.on_wait and len(si.on_wait) > limit:
                    waits = list(si.on_wait)
                    for wchunk in waits[:-limit]:
                        k += 1
                        new.append(mybir.InstNoOp(
                            name=f"wsplit-{k}-{inst.name}",
                            sync_info=mybir.SyncInfo(on_wait=[wchunk],
                                                     on_update=[]),
                            bass_nofuse=True,
                            engine=inst.engine,
                        ))
                    inst.sync_info = mybir.SyncInfo(on_wait=waits[-limit:],
                                                    on_update=si.on_update)
                new.append(inst)
            blk.instructions[:] = new


def _ensure_ntff_hook() -> bool:
    """The agent image lacks ``antenv.axon_hooks``; synthesize it and register
    the ctypes NTFF profile hook so trace=True works under axon."""
    try:
        from antenv.axon_hooks import get_axon_ntff_profile_hook
        return get_axon_ntff_profile_hook() is not None
    except ImportError:
        pass
    try:
        import types

        import antenv
        from trn_agent_boot.trn_boot import _ntff_profile_via_ctypes

        mod = types.ModuleType("antenv.axon_hooks")
        holder = {"hook": None}
        mod.set_axon_ntff_profile_hook = lambda h: holder.__setitem__("hook", h)
        mod.get_axon_ntff_profile_hook = lambda: holder["hook"]
        sys.modules["antenv.axon_hooks"] = mod
        antenv.axon_hooks = mod
        hook = _ntff_profile_via_ctypes("/opt/axon/libaxon_pjrt.so")
        if hook is None:
            return False
        mod.set_axon_ntff_profile_hook(hook)
        return True
    except Exception as e:  # pragma: no cover - profiling is best-effort
        print(f"NTFF hook setup failed: {e}", file=sys.stderr)
        return False


def _spectral_coeffs(S: np.ndarray, alpha: np.ndarray):
    """c_i, d_i with density(w) = sum_i c_i exp(d_i w).  Returns None if the
    eigendecomposition is complex/ill-conditioned (not the case for the
    phase-type sub-generators this problem builds)."""
    S64 = S.astype(np.float64)
    s_vec = -S64.sum(axis=1)
    try:
        d, V = np.linalg.eig(S64)
        c = (alpha.astype(np.float64) @ V) * np.linalg.solve(V, s_vec)
    except np.linalg.LinAlgError:
        return None
    if np.abs(d.imag).max() > 1e-8 or np.abs(c.imag).max() > 1e-6 * max(
            1.0, np.abs(c.real).max()):
        return None
    if not (np.isfinite(c.real).all() and np.isfinite(d.real).all()):
        return None
    return c.real.copy(), d.real.copy()


def _host_fallback(w, S, alpha):
    """Exact host computation for inputs outside the real-spectral fast path
    (complex eigenvalues / defective S).  Batched scaling-and-squaring expm
    in fp64 via numpy."""
    S64 = S.astype(np.float64)
    s_vec = -S64.sum(axis=1)
    w64 = w.astype(np.float64)
    n = S64.shape[0]
    A = w64[:, None, None] * S64          # [M, n, n]
    nrm = np.abs(A).sum(axis=2).max(axis=1)
    k = np.maximum(0, np.ceil(np.log2(np.maximum(nrm, 1e-300))) + 1).astype(int)
    kmax = int(k.max()) if len(k) else 0
    A = A / (2.0 ** k)[:, None, None]
    # Pade(7) approximant
    b = [17297280., 8648640., 1995840., 277200., 25200., 1512., 56., 1.]
    I = np.eye(n)
    A2 = A @ A
    A4 = A2 @ A2
    A6 = A4 @ A2
    U = A @ (b[7] * A6 + b[5] * A4 + b[3] * A2 + b[1] * I)
    Vp = b[6] * A6 + b[4] * A4 + b[2] * A2 + b[0] * I
    E = np.linalg.solve(Vp - U, Vp + U)
    for j in range(kmax):
        sel = k > j
        E[sel] = E[sel] @ E[sel]
    dens = np.einsum("i,mij,j->m", alpha.astype(np.float64), E, s_vec)
    return np.float32(np.log(dens).sum())


def _compress_terms(c, d, wmin, wmax, tol=1e-5, mean_tol=1e-7):
    """Refit the exponential sum sum_i c_i e^{d_i w} with as few terms as
    possible on [wmin, wmax] (exponential sums are extremely compressible:
    Hankel singular values decay geometrically).  Pointwise relative
    tolerance `tol`; returns the exact terms unchanged if no smaller fit
    qualifies.  Every ScalarE Exp activation removed saves ~0.7us."""
    n0 = len(c)
    if n0 <= 2 or wmax - wmin < 1e-9:
        return c, d
    try:
        from scipy.optimize import least_squares
    except ImportError:
        return c, d
    grid = np.linspace(wmin, wmax, 3001)
    target = np.exp(np.outer(grid, d)) @ c
    if target.min() <= 0 or not np.isfinite(target).all():
        return c, d
    order = np.argsort(-np.abs(c))
    for n in range(2, n0):
        idx = order[:n]
        x0 = np.concatenate([c[idx], d[idx]])

        def resid(x, n=n):
            return (np.exp(np.outer(grid, x[n:])) @ x[:n] - target) / target

        try:
            r = least_squares(resid, x0, method="lm", max_nfev=20000)
        except Exception:
            continue
        rr = resid(r.x)
        cf, df = r.x[:n].copy(), r.x[n:].copy()
        dens = np.exp(np.outer(grid, df)) @ cf
        # Pointwise error must stay at fp32-noise scale AND the mean residual
        # (what survives the 500k-element sum) must be ~zero.
        if (np.abs(rr).max() <= tol and abs(rr.mean()) <= mean_tol
                and np.isfinite(r.x).all()
                and dens.min() > 0 and np.abs(df).max() < 60):
            return cf, df
    return c, d


def _run_softplus(w: np.ndarray, fit) -> np.ndarray:
    """Execute the softplus-factored device program and assemble the total."""
    a, b, p, q = fit
    M = w.size
    per = -(-M // N_CORES)
    P = 128
    J = -(-per // P)
    shard = P * J
    PAD_VAL = 1.0
    n_pad_total = N_CORES * shard - M
    shards = []
    for i in range(N_CORES):
        lo = min(i * per, M)
        hi = min((i + 1) * per, M)
        wrow = np.empty(shard, np.float32)
        wrow[:hi - lo] = w[lo:hi]
        wrow[hi - lo:] = PAD_VAL
        shards.append(np.ascontiguousarray(
            wrow.reshape(P, J).astype(ml_dtypes.bfloat16)))

    key = ("softplus", p, q, P, J)
    nc = _program_cache.get(key)
    if nc is None:
        nc = _build_program_softplus(p, q, P, J)
        _program_cache[key] = nc

    in_maps = [{"wbf": shards[i]} for i in range(N_CORES)]
    trace = bool(os.environ.get("KERNEL_TRACE"))
    if trace:
        trace = _ensure_ntff_hook()
    res = run_bass_kernel_spmd(nc, in_maps, list(range(N_CORES)), trace=trace)
    global _last_results
    _last_results = res
    dev = 0.0
    for r in res.results:
        dev += r["partials"].astype(np.float64).sum()
    dev -= n_pad_total * float(np.exp(p + q * PAD_VAL))
    total = M * a + b * float(w.astype(np.float64).sum()) - dev
    return np.float32(total)


def kernel(w: np.ndarray, S: np.ndarray, alpha: np.ndarray) -> np.ndarray:
    w = np.ascontiguousarray(np.asarray(w).reshape(-1), dtype=np.float32)
    S = np.asarray(S, dtype=np.float32)
    alpha = np.asarray(alpha, dtype=np.float32)

    cd = _spectral_coeffs(S, alpha)
    if cd is None:
        return _host_fallback(w, S, alpha)
    c, d = cd
    # Fully-fused form: log dens ~ a + b*w - e^{p+q*w}.  The residual after
    # factoring the dominant exponential is itself nearly exponential, so a
    # single Exp activation (available in the PWP tables, unlike Softplus)
    # computes the whole per-element correction; a,b are exact on host.
    sp_fit = _fit_softplus(c, d, float(w.min()), float(w.max()))
    if sp_fit is not None:
        return _run_softplus(w, sp_fit)
    c, d = _compress_terms(c, d, float(w.min()), float(w.max()))
    # Drop numerically-zero terms.
    keep = np.abs(c) > 1e-300
    c, d = c[keep], d[keep]

    # Factor out the dominant positive term k:
    #   log dens = ln c_k + d_k w + ln(1 + sum_{i!=k} (c_i/c_k) e^{(d_i-d_k) w})
    # The affine part is an exact host-side reduction over the fp32 w (so it
    # carries none of the bf16 quantization the device input uses), and the
    # device computes one fewer exp plus a Ln whose +1 rides in the free
    # activation bias.
    affine0 = 0.0       # per-element constant (ln c_k)
    affine1 = 0.0       # per-element slope    (d_k)
    ln_bias_one = False
    ln_scale = 1.0
    if c.max() > 0 and len(c) >= 2:
        k = int(np.argmax(c))
        ck, dk = c[k], d[k]
        affine0, affine1 = float(np.log(ck)), float(dk)
        c = np.delete(c, k) / ck
        d = np.delete(d, k) - dk
        ln_bias_one = True
        if len(c) == 1:
            ln_scale = float(np.sign(c[0]))

    # Order NEGATIVES first (their add-chain then finishes early; see
    # _build_program).
    order = np.argsort(c > 0, kind="stable")
    c, d = c[order], d[order]
    n_neg = int((c < 0).sum()) if not (ln_bias_one and len(c) == 1) else 0
    logc = np.log(np.abs(c))

    M = w.size
    per = -(-M // N_CORES)          # ceil
    P = 128
    J = -(-per // P)                # ceil -> shard size P*J
    shard = P * J
    PAD_VAL = 1.0
    n_pad_total = N_CORES * shard - M
    shards = []
    for i in range(N_CORES):
        lo = min(i * per, M)
        hi = min((i + 1) * per, M)
        wrow = np.empty(shard, np.float32)
        wrow[:hi - lo] = w[lo:hi]
        wrow[hi - lo:] = PAD_VAL
        shards.append(np.ascontiguousarray(
            wrow.reshape(P, J).astype(ml_dtypes.bfloat16)))

    key = (d.tobytes(), logc.tobytes(), n_neg, P, J, ln_bias_one, ln_scale)
    nc = _program_cache.get(key)
    if nc is None:
        nc = _build_program(d, logc, n_neg, P, J,
                            ln_bias_one=ln_bias_one, ln_scale=ln_scale)
        _program_cache[key] = nc

    in_maps = [{"wbf": shards[i]} for i in range(N_CORES)]
    trace = bool(os.environ.get("KERNEL_TRACE"))
    if trace:
        trace = _ensure_ntff_hook()
    res = run_bass_kernel_spmd(nc, in_maps, list(range(N_CORES)), trace=trace)
    global _last_results
    _last_results = res
    total = 0.0
    for r in res.results:
        total += r["partials"].astype(np.float64).sum()
    # Remove the host-known padding contribution, then add the exact host
    # affine part (0 unless factored).
    pad_x = float(np.exp(d * PAD_VAL) @ c)
    pad_per = np.log1p(pad_x) if ln_bias_one else np.log(pad_x)
    if n_pad_total:
        total -= n_pad_total * float(pad_per)
    total += M * affine0 + affine1 * float(w.astype(np.float64).sum())
    return np.float32(total)


if __name__ == "__main__":
    z = np.load("/root/problem/inputs_cache.npz")
    out = kernel(z["w"], z["S"], z["alpha"])
    print("kernel output:", out)
